# revision 1
# baseline (speedup 1.0000x reference)
"""Trainium2 Bass kernel for DeepKernelRegressionModel.

Math (per core, X sharded by rows across 8 cores):
  Xf = MLP(X), Yf = MLP(Y)                        (3-layer relu MLP, H=32)
  K[i,m] = exp(-|Xf_i - Yf_m|^2 / 2)
         = exp(Xf_i . Yf_m - |Xf_i|^2/2 - |Yf_m|^2/2)
  out = (K @ Y_target) / (K @ 1)

Everything is fused: the exponent is produced by ONE tensor-engine matmul
with an augmented contraction dim (K=34):
  lhsT rows 0-31 = Yf^T, row 32 = 1,       row 33 = -|Yf|^2/2
  rhs  rows 0-31 = Xf^T, row 32 = -|Xf|^2/2, row 33 = 1
in the transposed orientation G'[m, i], so that the second matmul
  acc[t, i] += Z_chunk^T @ exp(G')      with Z = [Y_target, 1]
contracts over m (the partition dim) with no transposes of the big
exp matrix. A final tiny transpose + reciprocal produces out[i, t].

The MLPs run in the transposed orientation (features on partitions) with
4-way tile_position stacking so relu ops use all 128 partitions.
"""

import os
import numpy as np
from contextlib import ExitStack

import concourse.bass as bass
import concourse.tile as tile
from concourse import bacc, mybir

FP = mybir.dt.float32
FPR = mybir.dt.float32r
AF = mybir.ActivationFunctionType

D, H, T = 64, 32, 8
TZ = T + 1  # Y_target columns + ones column
ZP = 32     # Z padded to 32 cols so mm2 fully writes its PSUM stripes
N_CORES = 8


def _split_matmul_waits(nc):
    """Walrus's S3_LW lowering for self-loading (4-byte) matmuls supports only
    one sync-wait command. Move multi-waits onto a PE sequencer NoOp placed
    right before the matmul — the in-order NX applies them to the stream."""
    import bass_rust

    k = 0
    for fn in nc.m.functions:
        for blk in fn.blocks:
            out = []
            for inst in blk.instructions:
                si = inst.sync_info
                if (type(inst).__name__ == "InstMatmult" and si is not None
                        and si.on_wait and len(si.on_wait) >= 2):
                    waits = list(si.on_wait)
                    for w in waits[:-1]:
                        nop = mybir.InstNoOp(name=f"I-mmwait-{k}", ins=[],
                                             outs=[])
                        k += 1
                        nop.engine = inst.engine
                        nop.sync_info = bass_rust.SyncInfo(
                            on_wait=[w], on_update=[])
                        out.append(nop)
                    inst.sync_info = bass_rust.SyncInfo(
                        on_wait=[waits[-1]], on_update=list(si.on_update))
                out.append(inst)
            blk.instructions = out


def build_nc(n_sh, m_total, use_f32r=True, exp_group=3, split_waits=True):
    """Build the Bass program for one core (SPMD: same program, all cores).

    n_sh: rows of X handled by this core. m_total: rows of Y (full).
    """
    assert n_sh % 512 == 0 and m_total % 2048 == 0
    MT = m_total // 128       # number of 128-row m-tiles
    NCH = m_total // 512      # number of 512-wide m-chunks (MLP)
    XG = n_sh // 4            # X stacked-chunk width
    IC = n_sh // 512          # i-chunks
    ICW = 512

    def r(ap):
        return ap.bitcast(FPR) if use_f32r else ap

    nc = bacc.Bacc("TRN2", target_bir_lowering=False, debug=False,
                   num_devices=N_CORES)

    Xd = nc.dram_tensor("X", [n_sh, D], FP, kind="ExternalInput").ap()
    Yd = nc.dram_tensor("Y", [m_total, D], FP, kind="ExternalInput").ap()
    Zd = nc.dram_tensor("Zm", [m_total, ZP], FP, kind="ExternalInput").ap()
    W1d = nc.dram_tensor("W1", [D, H], FP, kind="ExternalInput").ap()
    W2d = nc.dram_tensor("W2", [H, H], FP, kind="ExternalInput").ap()
    W3d = nc.dram_tensor("W3", [H, H], FP, kind="ExternalInput").ap()
    Bd = nc.dram_tensor("Bs", [128, 3], FP, kind="ExternalInput").ap()
    Id = nc.dram_tensor("ident", [128, 128], FP, kind="ExternalInput").ap()
    NHd = nc.dram_tensor("neghalf", [128, 32], FP, kind="ExternalInput").ap()
    ORd = nc.dram_tensor("onesrow", [1, m_total], FP, kind="ExternalInput").ap()
    OUTd = nc.dram_tensor("out", [n_sh, T], FP, kind="ExternalOutput").ap()

    with tile.TileContext(nc) as tc, ExitStack() as ctx:
        const = ctx.enter_context(tc.tile_pool(name="const", bufs=1))
        big = ctx.enter_context(tc.tile_pool(name="big", bufs=1))
        scr = ctx.enter_context(tc.tile_pool(name="scr", bufs=1))

        w1s = const.tile([D, H], FP)
        nc.sync.dma_start(w1s[:], W1d[:])
        w2s = const.tile([128, H], FP)
        w3s = const.tile([128, H], FP)
        for g in range(4):
            nc.sync.dma_start(w2s[32 * g:32 * g + 32, :], W2d[:])
            nc.sync.dma_start(w3s[32 * g:32 * g + 32, :], W3d[:])
        bs = const.tile([128, 3], FP)
        nc.sync.dma_start(bs[:], Bd[:])
        ident = const.tile([128, 128], FP)
        nc.sync.dma_start(ident[:], Id[:])
        nh = const.tile([128, 32], FP)
        nc.sync.dma_start(nh[:], NHd[:])
        zt = const.tile([128, MT * ZP], FP)
        nc.sync.dma_start(
            r(zt.rearrange("p (t c) -> p t c", c=ZP)),
            r(Zd.rearrange("(t p) c -> p t c", p=128)),
        )

        # persistent big tensors
        yT = big.tile([D, m_total], FP)      # Y^T
        xT = big.tile([D, n_sh], FP)         # X^T
        yft = big.tile([128, m_total], FP)   # rows 0-33 aug A, 64-97 aug B
        xft = big.tile([128, n_sh], FP)

        # ---------------- phase A: transposes (PE) ----------------
        with (
            tc.tile_pool(name="tp_psum", bufs=2, space="PSUM") as tpp,
            tc.tile_pool(name="ytile", bufs=4) as ytp,
        ):
            n_ych = (MT + 7) // 8
            for c in range(n_ych):
                ts = list(range(8 * c, min(8 * c + 8, MT)))
                tp = tpp.tile([D, 128 * len(ts)], FP, tag="tp")
                for k, mt in enumerate(ts):
                    ytile = ytp.tile([128, D], FP, tag="yt")
                    nc.sync.dma_start(ytile[:], Yd[128 * mt:128 * mt + 128, :])
                    nc.tensor.transpose(tp[:, 128 * k:128 * k + 128],
                                        ytile[:], ident[:])
                nc.vector.tensor_copy(
                    yT[:, 1024 * c:1024 * c + 128 * len(ts)], tp[:])
            n_xch = (n_sh // 128 + 7) // 8
            for c in range(n_xch):
                ts = list(range(8 * c, min(8 * c + 8, n_sh // 128)))
                tp = tpp.tile([D, 128 * len(ts)], FP, tag="tp")
                for k, mt in enumerate(ts):
                    xtile = ytp.tile([128, D], FP, tag="yt")
                    nc.sync.dma_start(xtile[:], Xd[128 * mt:128 * mt + 128, :])
                    nc.tensor.transpose(tp[:, 128 * k:128 * k + 128],
                                        xtile[:], ident[:])
                nc.vector.tensor_copy(
                    xT[:, 1024 * c:1024 * c + 128 * len(ts)], tp[:])

        # ---------------- phase B: Y MLP (stacked 4x) ----------------
        # chunk ch (512 m's) -> partition group cg = ch%4, col chunk cc = ch//4
        CCY = NCH // 4
        yfp = ctx.enter_context(tc.tile_pool(name="yf_pool", bufs=1))
        with (
            tc.tile_pool(name="mlp_psum", bufs=2, space="PSUM") as mpp,
            tc.tile_pool(name="acts", bufs=2) as actp,
        ):
            h1p = mpp.tile([128, 512 * CCY], FP, tag="hp")
            for ch in range(NCH):
                cg, cc = ch % 4, ch // 4
                nc.tensor.matmul(h1p[32 * cg:32 * cg + 32, 512 * cc:512 * cc + 512],
                                 lhsT=w1s[:], rhs=yT[:, 512 * ch:512 * ch + 512],
                                 start=True, stop=True,
                                 skip_group_check=True,
                                 tile_position=(0, 32 * cg))
            h1s = actp.tile([128, 512 * CCY], FP, tag="hs")
            for cc in range(CCY):
                nc.scalar.activation(h1s[:, 512 * cc:512 * cc + 512],
                                     h1p[:, 512 * cc:512 * cc + 512],
                                     AF.Relu, bias=bs[:, 0:1])
            h2p = mpp.tile([128, 512 * CCY], FP, tag="hp")
            for ch in range(NCH):
                cg, cc = ch % 4, ch // 4
                nc.tensor.matmul(h2p[32 * cg:32 * cg + 32, 512 * cc:512 * cc + 512],
                                 tile_position=(32 * cg, 32 * cg),
                                 lhsT=w2s[32 * cg:32 * cg + 32, :],
                                 rhs=h1s[32 * cg:32 * cg + 32, 512 * cc:512 * cc + 512],
                                 start=True, stop=True,
                                 skip_group_check=True)
            h2s = actp.tile([128, 512 * CCY], FP, tag="hs")
            for cc in range(CCY):
                nc.scalar.activation(h2s[:, 512 * cc:512 * cc + 512],
                                     h2p[:, 512 * cc:512 * cc + 512],
                                     AF.Relu, bias=bs[:, 1:2])
            h3p = mpp.tile([128, 512 * CCY], FP, tag="hp")
            for ch in range(NCH):
                cg, cc = ch % 4, ch // 4
                nc.tensor.matmul(h3p[32 * cg:32 * cg + 32, 512 * cc:512 * cc + 512],
                                 tile_position=(32 * cg, 32 * cg),
                                 lhsT=w3s[32 * cg:32 * cg + 32, :],
                                 rhs=h2s[32 * cg:32 * cg + 32, 512 * cc:512 * cc + 512],
                                 start=True, stop=True,
                                 skip_group_check=True)
            yfs = yfp.tile([128, 512 * CCY], FP, tag="yfs")
            sqy = yfp.tile([128, 512 * CCY], FP, tag="sqy")
            for cc in range(CCY):
                nc.scalar.activation(r(yfs[:, 512 * cc:512 * cc + 512]),
                                     h3p[:, 512 * cc:512 * cc + 512],
                                     AF.Relu, bias=bs[:, 2:3])
                nc.vector.tensor_mul(sqy[:, 512 * cc:512 * cc + 512],
                                     yfs[:, 512 * cc:512 * cc + 512],
                                     yfs[:, 512 * cc:512 * cc + 512])
            # assemble yft rows 0-31 (flat layout)
            for ch in range(NCH):
                cg, cc = ch % 4, ch // 4
                nc.sync.dma_start(r(yft[0:32, 512 * ch:512 * ch + 512]),
                                  r(yfs[32 * cg:32 * cg + 32, 512 * cc:512 * cc + 512]))
            nc.sync.dma_start(r(yft[32:33, :]), r(ORd[:]))  # ones row

        # ---------------- phase C: norms + X MLP ----------------
        with (
            tc.tile_pool(name="ynp", bufs=2, space="PSUM") as ynpp,
            tc.tile_pool(name="xnp", bufs=1, space="PSUM") as xnpp,
        ):
            # ynorm row: -|Yf_m|^2/2 for every m, laid out on partitions
            # {0,32,64,96} x 1024 cols per psum tile (one tile per 4096 m)
            n_yn = (NCH + 7) // 8
            for a in range(n_yn):
                chs = list(range(8 * a, min(8 * a + 8, NCH)))
                ynp = ynpp.tile([128, 1024], FP, tag="ynp")
                for chl, ch in enumerate(chs):
                    cg, cc = ch % 4, ch // 4
                    prow, pcol = 32 * (chl // 2), 512 * (chl % 2)
                    nc.tensor.matmul(ynp[prow:prow + 32, pcol:pcol + 512],
                                     tile_position=(32 * cg, prow),
                                     lhsT=nh[32 * cg:32 * cg + 32, :],
                                     rhs=sqy[32 * cg:32 * cg + 32,
                                             512 * cc:512 * cc + 512],
                                     start=True, stop=True,
                                     skip_group_check=True)
                yns = scr.tile([128, 1024], FP, tag="yns")
                nrow = 32 * ((len(chs) + 1) // 2)
                nc.vector.tensor_copy(r(yns[0:nrow, :]), ynp[0:nrow, :])
                for k in range(len(chs) // 2):
                    nc.sync.dma_start(
                        r(yft[33:34, 4096 * a + 1024 * k:4096 * a + 1024 * k + 1024]),
                        r(yns[32 * k:32 * k + 1, :]))
            # duplicate augmented block to partitions 64-97 (row group B)
            for sg in range(4):
                seg = m_total // 4
                nc.sync.dma_start(r(yft[64:98, seg * sg:seg * sg + seg]),
                                  r(yft[0:34, seg * sg:seg * sg + seg]))

            # ---- X MLP (4 chunks of XG cols, stacked) ----
            hx1 = xnpp.tile([128, XG], FP, tag="hx")
            for ch in range(4):
                nc.tensor.matmul(hx1[32 * ch:32 * ch + 32, :],
                                 tile_position=(0, 32 * ch),
                                 lhsT=w1s[:],
                                 rhs=xT[:, XG * ch:XG * ch + XG],
                                 start=True, stop=True,
                                 skip_group_check=True)
            hx1s = scr.tile([128, XG], FP, tag="hxs1")
            nc.scalar.activation(hx1s[:], hx1[:], AF.Relu, bias=bs[:, 0:1])
            hx2 = xnpp.tile([128, XG], FP, tag="hx")
            for ch in range(4):
                nc.tensor.matmul(hx2[32 * ch:32 * ch + 32, :],
                                 tile_position=(32 * ch, 32 * ch),
                                 lhsT=w2s[32 * ch:32 * ch + 32, :],
                                 rhs=hx1s[32 * ch:32 * ch + 32, :],
                                 start=True, stop=True,
                                 skip_group_check=True)
            hx2s = scr.tile([128, XG], FP, tag="hxs2")
            nc.scalar.activation(hx2s[:], hx2[:], AF.Relu, bias=bs[:, 1:2])
            hx3 = xnpp.tile([128, XG], FP, tag="hx")
            for ch in range(4):
                nc.tensor.matmul(hx3[32 * ch:32 * ch + 32, :],
                                 tile_position=(32 * ch, 32 * ch),
                                 lhsT=w3s[32 * ch:32 * ch + 32, :],
                                 rhs=hx2s[32 * ch:32 * ch + 32, :],
                                 start=True, stop=True,
                                 skip_group_check=True)
            xfs = scr.tile([128, XG], FP, tag="xfs")
            nc.scalar.activation(r(xfs[:]), hx3[:], AF.Relu, bias=bs[:, 2:3])
            sqx = scr.tile([128, XG], FP, tag="sqx")
            nc.vector.tensor_mul(sqx[:], xfs[:], xfs[:])
            for ch in range(4):
                nc.sync.dma_start(r(xft[0:32, XG * ch:XG * ch + XG]),
                                  r(xfs[32 * ch:32 * ch + 32, :]))
            nc.sync.dma_start(r(xft[33:34, :]), r(ORd[0:1, 0:n_sh]))  # ones row
            xnp = xnpp.tile([128, XG], FP, tag="xnp")
            for ch in range(4):
                nc.tensor.matmul(xnp[32 * ch:32 * ch + 32, :],
                                 tile_position=(32 * ch, 32 * ch),
                                 lhsT=nh[32 * ch:32 * ch + 32, :],
                                 rhs=sqx[32 * ch:32 * ch + 32, :],
                                 start=True, stop=True,
                                 skip_group_check=True)
            xns = scr.tile([128, XG], FP, tag="xns")
            nc.vector.tensor_copy(r(xns[:]), xnp[:])
            for ch in range(4):
                nc.sync.dma_start(r(xft[32:33, XG * ch:XG * ch + XG]),
                                  r(xns[32 * ch:32 * ch + 1, :]))
            nc.sync.dma_start(r(xft[64:98, :]), r(xft[0:34, :]))

        # ---------------- main loop ----------------
        groups = []
        mt = 0
        while mt < MT:
            groups.append(list(range(mt, min(mt + exp_group, MT))))
            mt += exp_group

        with (
            tc.tile_pool(name="gbuf", bufs=2, space="PSUM") as gpool,
            tc.tile_pool(name="accp", bufs=2, space="PSUM") as apool,
            tc.tile_pool(name="ebuf", bufs=3) as epool,
            tc.tile_pool(name="fin", bufs=2) as finp,
        ):
            for ic in range(IC):
                acc = apool.tile([128, ICW], FP, tag="acc")
                for grp in groups:
                    gp = gpool.tile([128, 512 * exp_group], FP, tag="g")
                    for t, mt in enumerate(grp):
                        rg = 64 * (mt % 2)
                        nc.tensor.matmul(
                            gp[:, 512 * t:512 * t + 512],
                            tile_position=(rg, 0),
                            lhsT=r(yft[rg:rg + 34, 128 * mt:128 * mt + 128]),
                            rhs=r(xft[rg:rg + 34, ICW * ic:ICW * ic + ICW]),
                            start=True, stop=True)
                    eb = epool.tile([128, 512 * exp_group], FP, tag="e")
                    w = 512 * len(grp)
                    nc.scalar.activation(r(eb[:, :w]), gp[:, :w], AF.Exp)
                    for t, mt in enumerate(grp):
                        nc.tensor.matmul(
                            acc[0:32, :],
                            tile_position=(0, 0),
                            lhsT=r(zt[:, ZP * mt:ZP * mt + ZP]),
                            rhs=r(eb[:, 512 * t:512 * t + 512]),
                            start=(mt == 0), stop=(mt == MT - 1),
                            skip_group_check=True)
                # fold 4 col-group accumulators via transpose-accumulate
                acc_s = finp.tile([32, ICW], FP, tag="accs")
                nc.vector.tensor_copy(acc_s[:], acc[0:32, :])
                ot = apool.tile([128, 128], FP, tag="acc")
                for q in range(4):
                    nc.tensor.matmul(
                        ot[:, 32 * q:32 * q + 32],
                        tile_position=(0, 0),
                        lhsT=acc_s[0:32, 128 * q:128 * q + 128],
                        rhs=ident[0:32, 0:32],
                        is_transpose=True,
                        start=(q == 0), stop=(q == 3),
                        skip_group_check=True)
                for q in range(4):
                    rec = finp.tile([128, 1], FP, tag="rec")
                    nc.vector.reciprocal(rec[:], ot[:, 32 * q + T:32 * q + T + 1])
                    res = finp.tile([128, T], FP, tag="res")
                    nc.vector.tensor_scalar_mul(res[:], ot[:, 32 * q:32 * q + T],
                                                rec[:])
                    nc.sync.dma_start(
                        OUTd[ICW * ic + 128 * q:ICW * ic + 128 * q + 128, :],
                        res[:])
    nc.compile()
    return nc


def make_in_maps(X, Y, Y_target, W1, b1, W2, b2, W3, b3, n_cores=N_CORES):
    f = lambda a: np.ascontiguousarray(np.asarray(a, dtype=np.float32))
    X, Y, Y_target = f(X), f(Y), f(Y_target)
    W1, W2, W3 = f(W1), f(W2), f(W3)
    b1, b2, b3 = f(b1), f(b2), f(b3)
    m_total = Y.shape[0]
    n_sh = X.shape[0] // n_cores
    Zm = np.zeros((m_total, 32), np.float32)
    Zm[:, :T] = Y_target
    Zm[:, T] = 1.0
    Bs = np.stack([np.tile(b1, 4), np.tile(b2, 4), np.tile(b3, 4)], axis=1)
    common = dict(
        Y=Y, Zm=Zm, W1=W1, W2=W2, W3=W3,
        Bs=np.ascontiguousarray(Bs),
        ident=np.eye(128, dtype=np.float32),
        neghalf=np.full((128, 32), -0.5, np.float32),
        onesrow=np.ones((1, m_total), np.float32),
    )
    return [dict(common, X=X[c * n_sh:(c + 1) * n_sh]) for c in range(n_cores)]


_NC_CACHE = {}


def _get_nc(n_sh, m_total):
    key = (n_sh, m_total)
    if key not in _NC_CACHE:
        use_f32r = os.environ.get("DKR_F32R", "1") == "1"
        _NC_CACHE[key] = build_nc(n_sh, m_total, use_f32r=use_f32r)
    return _NC_CACHE[key]


def kernel(X, Y, Y_target, W1, b1, W2, b2, W3, b3):
    from concourse.bass_utils import run_bass_kernel_spmd

    in_maps = make_in_maps(X, Y, Y_target, W1, b1, W2, b2, W3, b3)
    n_sh = in_maps[0]["X"].shape[0]
    nc = _get_nc(n_sh, np.asarray(Y).shape[0])
    res = run_bass_kernel_spmd(nc, in_maps, core_ids=list(range(N_CORES)))
    return np.concatenate([res.results[c]["out"] for c in range(N_CORES)], axis=0)



# revision 20
# speedup vs baseline: 3231.9340x; 3231.9340x over previous
"""Trainium2 Bass kernel for DeepKernelRegressionModel (v2).

Math (per core, X sharded by rows across 8 cores):
  Xf = MLP(X), Yf = MLP(Y)                 (3-layer relu MLP, H=32)
  K[i,m] = exp(Xf_i . Yf_m - |Yf_m|^2/2)   (x-norm term cancels in the
                                            normalized ratio, so skip it)
  out = (K @ Y_target) / (K @ 1)

Design:
  - Y loaded in 8 batched DMAs as [128, 512] tiles, PE-transposed two
    m-tiles per transpose into yT [128, 4096] (even tiles on partitions
    0-63, odd on 64-127).
  - Y-MLP runs 4-way stacked; its relu output yfs [128, 2048] is read
    DIRECTLY as mm1's stationary operand (no assembled yft, no SP DMA
    assembly).
  - y-norms are computed as per-m-tile [128,1] bias COLUMNS via tiny
    matmuls (sqy-slice^T @ -0.5) and applied as the exp activation bias.
  - exp engine per m-tile is configurable: 'A' = exact exp on ACT (f32
    output), 'P'/'D' = Schraudolph bf16 bit-trick on Pool/DVE (one
    tensor_scalar op writing int16 bits of a bf16 exp approximation).
  - mm2 contracts exp tiles with Z = [Y_target, 1, pad] (ZP=16 cols) in
    matching dtype (f32r or bf16); m-tiles are visited in an order that
    rotates mm1 across all 4 PE row-groups for tile concurrency.
"""

import os
import numpy as np
from contextlib import ExitStack

import concourse.bass as bass
import concourse.tile as tile
from concourse import bacc, mybir

FP = mybir.dt.float32
FPR = mybir.dt.float32r
BF = mybir.dt.bfloat16
I16 = mybir.dt.int16
AF = mybir.ActivationFunctionType
ALU = mybir.AluOpType

D, H, T = 64, 32, 8
ZP = 16          # Z columns: Y_target(8) + ones + pad
N_CORES = 8

LN2 = 0.6931471805599453
EXP_S = 128.0 / LN2          # bf16 schraudolph scale
SIGMA = 0.058
EXP_B = (127.0 - SIGMA) * 128.0

# exp-engine pattern over reordered m-tile position (period 8):
# 'A' exact ACT, 'P' Pool bit-trick, 'D' DVE bit-trick
PATTERN = os.environ.get("DKR_PATTERN", "AAAAAAAA")


def mt_order(MT):
    """Visit order rotating mm1 row-groups 0,1,2,3. Octet pair (16 tiles):
    [16a+2j, 16a+2j+1, 16a+8+2j, 16a+8+2j+1] has cg 0,1,2,3."""
    order = []
    a = 0
    while 16 * a < MT:
        hi = 16 * a + 8 < MT
        for j in range(4):
            order.append(16 * a + 2 * j)
            order.append(16 * a + 2 * j + 1)
            if hi:
                order.append(16 * a + 8 + 2 * j)
                order.append(16 * a + 8 + 2 * j + 1)
        a += 1
    assert sorted(order) == list(range(MT))
    return order


def build_nc(n_sh, m_total, use_f32r=True, pattern=None, iters=1,
             split_waits=True):
    assert n_sh % 1024 == 0 and m_total % 2048 == 0
    MT = m_total // 128     # m-tiles
    NCH = m_total // 512    # MLP chunks
    CCY = NCH // 4
    NYD = m_total // 1024   # batched Y DMAs
    IC = n_sh // 512
    ICW = 512
    NXT = n_sh // 128       # x tiles
    pattern = pattern or PATTERN

    def r(ap):
        return ap.bitcast(FPR) if use_f32r else ap

    nc = bacc.Bacc("TRN2", target_bir_lowering=False, debug=False,
                   num_devices=N_CORES)

    Xd = nc.dram_tensor("X", [n_sh, D], FP, kind="ExternalInput").ap()
    Yd = nc.dram_tensor("Y", [m_total, D], FP, kind="ExternalInput").ap()
    Zfd = nc.dram_tensor("Zf", [m_total, ZP], FP, kind="ExternalInput").ap()
    Zbd = nc.dram_tensor("Zb", [m_total, ZP], BF, kind="ExternalInput").ap()
    W1d = nc.dram_tensor("W1", [D, H], FP, kind="ExternalInput").ap()
    W2d = nc.dram_tensor("W2", [H, H], FP, kind="ExternalInput").ap()
    W3d = nc.dram_tensor("W3", [H, H], FP, kind="ExternalInput").ap()
    Bd = nc.dram_tensor("Bs", [128, 3], FP, kind="ExternalInput").ap()
    Id = nc.dram_tensor("ident", [128, 128], FP, kind="ExternalInput").ap()
    OUTd = nc.dram_tensor("out", [n_sh, T], FP, kind="ExternalOutput").ap()

    with tile.TileContext(nc) as tc, ExitStack() as octx:
        loop_cm = tc.For_i(0, iters, name="bench") if iters > 1 else None
        if loop_cm is not None:
            octx.enter_context(loop_cm)
        with ExitStack() as ctx:
            const = ctx.enter_context(tc.tile_pool(name="const", bufs=1))
            big = ctx.enter_context(tc.tile_pool(name="big", bufs=1))

            w1s = const.tile([128, H], FP)
            nc.sync.dma_start(r(w1s[0:D, :]), r(W1d[:]))
            nc.sync.dma_start(r(w1s[D:128, :]), r(W1d[:]))
            w2s = const.tile([128, H], FP)
            w3s = const.tile([128, H], FP)
            for g in range(4):
                nc.sync.dma_start(r(w2s[32 * g:32 * g + 32, :]), r(W2d[:]))
                nc.sync.dma_start(r(w3s[32 * g:32 * g + 32, :]), r(W3d[:]))
            bs = const.tile([128, 3], FP)
            nc.sync.dma_start(bs[:], Bd[:])
            ident = const.tile([128, 128], FP)
            nc.sync.dma_start(r(ident[:]), r(Id[:]))
            nh = const.tile([128, 1], FP)
            nc.gpsimd.memset(nh[:], -0.5)

            ztf = const.tile([128, MT * ZP], FP)
            nc.scalar.dma_start(
                r(ztf.rearrange("p (t c) -> p t c", c=ZP)),
                r(Zfd.rearrange("(t p) c -> p t c", p=128)),
            )
            ztb = const.tile([128, MT * ZP], BF)
            nc.scalar.dma_start(
                ztb.rearrange("p (t c) -> p t c", c=ZP),
                Zbd.rearrange("(t p) c -> p t c", p=128),
            )

            yT = big.tile([128, m_total // 2], FP)   # packed transposed Y
            xT = big.tile([128, n_sh // 2], FP)
            yfs = big.tile([128, m_total // 4], FP)  # MLP(Y)^T, 4-way stacked
            xft = big.tile([128, n_sh], FP)          # MLP(X)^T, replicated x4
            ynb = big.tile([128, MT], FP)            # -|Yf|^2/2 bias columns
            ynb2 = big.tile([128, MT], FP)           # scaled for bit-trick

            # ---------- phase A: load + transpose (+ X MLP early) ----------
            with (
                tc.tile_pool(name="tp_psum", bufs=2, space="PSUM") as tpp,
                tc.tile_pool(name="raw", bufs=2) as rawp,
                tc.tile_pool(name="xp", bufs=2, space="PSUM") as xpp,
                tc.tile_pool(name="xp3", bufs=1, space="PSUM") as xpp3,
                tc.tile_pool(name="xacts", bufs=2) as xactp,
            ):
                xraw = rawp.tile([128, 512], FP, tag="xraw")
                nc.sync.dma_start(
                    r(xraw.rearrange("p (t c) -> p t c", c=D)),
                    r(Xd.rearrange("(t p) c -> p t c", p=128)),
                )
                tp = tpp.tile([128, 512], FP, tag="tp")
                for j in range(4):
                    nc.tensor.transpose(r(tp[:, 128 * j:128 * j + 128]),
                                        r(xraw[:, 128 * j:128 * j + 128]),
                                        r(ident[:]))
                nc.vector.tensor_copy(r(xT[:]), tp[:])

                for g in range(NYD // 2):
                    yraw = rawp.tile([128, 1024], FP, tag="raw")
                    nc.sync.dma_start(
                        r(yraw.rearrange("p (t c) -> p t c", c=D)),
                        r(Yd[2048 * g:2048 * g + 2048, :].rearrange(
                            "(t p) c -> p t c", p=128)),
                    )
                    for h in range(2):
                        tp = tpp.tile([128, 512], FP, tag="tp")
                        for j in range(4):
                            nc.tensor.transpose(
                                r(tp[:, 128 * j:128 * j + 128]),
                                r(yraw[:, 512 * h + 128 * j:
                                        512 * h + 128 * j + 128]),
                                r(ident[:]))
                        nc.vector.tensor_copy(
                            r(yT[:, 1024 * g + 512 * h:1024 * g + 512 * h + 512]),
                            tp[:])

                # X MLP (f32r, flat rows 0-31), interleaved with Y loads
                hx1 = xpp.tile([H, n_sh], FP, tag="hx")
                for half in range(2):
                    nc.tensor.matmul(
                        hx1[0:32, 512 * half:512 * half + 512],
                        tile_position=(64 * half, 0),
                        lhsT=r(w1s[64 * half:64 * half + 64, :]),
                        rhs=r(xT[64 * half:64 * half + 64, :]),
                        start=True, stop=True, skip_group_check=True)
                hx1s = xactp.tile([H, n_sh], FP, tag="hxs")
                nc.scalar.activation(r(hx1s[:]), hx1[:], AF.Relu,
                                      bias=bs[0:H, 0:1])
                hx2 = xpp.tile([H, n_sh], FP, tag="hx")
                for half in range(2):
                    nc.tensor.matmul(
                        hx2[0:32, 512 * half:512 * half + 512],
                        tile_position=(0, 0),
                        lhsT=r(w2s[0:32, :]),
                        rhs=r(hx1s[0:32, 512 * half:512 * half + 512]),
                        start=True, stop=True, skip_group_check=True)
                hx2s = xactp.tile([H, n_sh], FP, tag="hxs")
                nc.vector.tensor_scalar(r(hx2s[:]), hx2[:], bs[0:H, 1:2], 0.0,
                                        op0=ALU.add, op1=ALU.max)
                hx3 = xpp3.tile([H, n_sh], FP, tag="hx3")
                for half in range(2):
                    nc.tensor.matmul(
                        hx3[0:32, 512 * half:512 * half + 512],
                        tile_position=(0, 0),
                        lhsT=r(w3s[0:32, :]),
                        rhs=r(hx2s[0:32, 512 * half:512 * half + 512]),
                        start=True, stop=True, skip_group_check=True)
                nc.vector.tensor_scalar(r(xft[0:32, :]), hx3[0:32, :],
                                        bs[0:H, 2:3], 0.0,
                                        op0=ALU.add, op1=ALU.max)
                for gg in range(1, 4):
                    nc.gpsimd.dma_start(r(xft[32 * gg:32 * gg + 32, :]),
                                        r(xft[0:32, :]))

            # ---------- phase B: Y MLP ----------
            # L1/L2 are f32r, which the ISA only allows at column-group 0,
            # so they emit flat [32, m] rows 0-31. L3 is plain fp32 (legal
            # with column groups) and emits the 4-way partition-stacked yfs
            # that mm1's rotating row-groups read directly.
            sqyp = ctx.enter_context(tc.tile_pool(name="sqy", bufs=1))
            with (
                tc.tile_pool(name="mlp_psum", bufs=2, space="PSUM") as mpp,
                tc.tile_pool(name="l3_psum", bufs=2, space="PSUM") as mpp3,
                tc.tile_pool(name="acts", bufs=1) as actp,
            ):
                h1s = actp.tile([H, m_total], FP, tag="h1s")
                h2s = actp.tile([H, m_total], FP, tag="h2s")
                npass = (NCH + 1) // 2
                for p in range(npass):
                    chs = range(2 * p, min(2 * p + 2, NCH))
                    h1p = mpp.tile([H, 1024], FP, tag="hp")
                    for i, ch in enumerate(chs):
                        q, half = ch // 2, ch % 2
                        nc.tensor.matmul(
                            h1p[:, 512 * i:512 * i + 512],
                            lhsT=r(w1s[64 * half:64 * half + 64, :]),
                            rhs=r(yT[64 * half:64 * half + 64,
                                     512 * q:512 * q + 512]),
                            tile_position=(64 * half, 0),
                            start=True, stop=True, skip_group_check=True)
                    nc.scalar.activation(
                        r(h1s[:, 1024 * p:1024 * p + 512 * len(chs)]),
                        h1p[:, 0:512 * len(chs)], AF.Relu, bias=bs[0:H, 0:1])
                for p in range(npass):
                    chs = range(2 * p, min(2 * p + 2, NCH))
                    h2p = mpp.tile([H, 1024], FP, tag="hp")
                    for i, ch in enumerate(chs):
                        nc.tensor.matmul(
                            h2p[:, 512 * i:512 * i + 512],
                            lhsT=r(w2s[0:32, :]),
                            rhs=r(h1s[0:32, 512 * ch:512 * ch + 512]),
                            tile_position=(0, 0),
                            start=True, stop=True, skip_group_check=True)
                    nc.vector.tensor_scalar(
                        r(h2s[:, 1024 * p:1024 * p + 512 * len(chs)]),
                        h2p[:, 0:512 * len(chs)], bs[0:H, 1:2], 0.0,
                        op0=ALU.add, op1=ALU.max)
                # L3: fp32, col-grouped into the stacked layout, per-cc
                sqy = sqyp.tile([128, 512 * CCY], FP, tag="sqy")
                for cc in range(CCY):
                    h3p = mpp3.tile([128, 512], FP, tag="h3p")
                    for cg in range(4):
                        ch = 4 * cc + cg
                        nc.tensor.matmul(
                            h3p[32 * cg:32 * cg + 32, :],
                            tile_position=(0, 32 * cg),
                            lhsT=w3s[0:32, :],
                            rhs=h2s[0:32, 512 * ch:512 * ch + 512],
                            start=True, stop=True, skip_group_check=True)
                    nc.vector.tensor_scalar(
                        r(yfs[:, 512 * cc:512 * cc + 512]),
                        h3p[:], bs[:, 2:3], 0.0, op0=ALU.add, op1=ALU.max)
                    nc.vector.tensor_mul(sqy[:, 512 * cc:512 * cc + 512],
                                         yfs[:, 512 * cc:512 * cc + 512],
                                         yfs[:, 512 * cc:512 * cc + 512])

            def yfs_slice(mt):
                ch = 2 * (mt // 8) + (mt % 8) % 2
                j = (mt % 8) // 2
                cg, cc = ch % 4, ch // 4
                col = 512 * cc + 128 * j
                return cg, yfs[32 * cg:32 * cg + 32, col:col + 128]

            def sqy_slice(mt):
                ch = 2 * (mt // 8) + (mt % 8) % 2
                j = (mt % 8) // 2
                cg, cc = ch % 4, ch // 4
                col = 512 * cc + 128 * j
                return cg, sqy[32 * cg:32 * cg + 32, col:col + 128]

            # ---------- phase C: norm columns + X MLP ----------
            with (
                tc.tile_pool(name="ynp", bufs=2, space="PSUM") as ynpp,
            ):
                nyb = (MT + 15) // 16
                for b in range(nyb):
                    ynp = ynpp.tile([128, 16], FP, tag="ynp")
                    mts = list(range(16 * b, min(16 * b + 16, MT)))
                    for k, mt in enumerate(mts):
                        cg, sl = sqy_slice(mt)
                        nc.tensor.matmul(
                            ynp[:, k:k + 1],
                            tile_position=(32 * cg, 0),
                            lhsT=sl, rhs=nh[32 * cg:32 * cg + 32, :],
                            start=True, stop=True, skip_group_check=True)
                    nc.vector.tensor_copy(ynb[:, 16 * b:16 * b + len(mts)],
                                          ynp[:, 0:len(mts)])
                nc.vector.tensor_scalar(ynb2[:], ynb[:], float(EXP_S),
                                        float(EXP_B), op0=ALU.mult,
                                        op1=ALU.add)

            # ---------- main loop ----------
            # Both i-chunks processed per m-tile: one [128, 1024] exp per
            # tile (single bias), one weight load per mm1 pair, and two
            # long-lived accumulators.
            order = mt_order(MT)
            with (
                tc.tile_pool(name="gbuf", bufs=2, space="PSUM") as gpool,
                tc.tile_pool(name="accp", bufs=1, space="PSUM") as apool,
                tc.tile_pool(name="ebuf", bufs=3) as epool,
                tc.tile_pool(name="fin", bufs=2) as finp,
            ):
                accs = []
                for ic in range(IC):
                    acc = apool.tile([128, ICW], FP, tag=f"acc{ic}")
                    accs.append(acc)
                for k, mt in enumerate(order):
                    eng = pattern[k % len(pattern)]
                    cg, ysl = yfs_slice(mt)
                    gp = gpool.tile([128, IC * ICW], FP, tag="g")
                    for ic in range(IC):
                        nc.tensor.matmul(
                            gp[:, ICW * ic:ICW * ic + ICW],
                            tile_position=(32 * cg, 0),
                            lhsT=r(ysl),
                            rhs=r(xft[32 * cg:32 * cg + 32,
                                      ICW * ic:ICW * ic + ICW]),
                            start=True, stop=True, skip_group_check=True)
                    if eng == "A":
                        eb = epool.tile([128, IC * ICW], FP, tag="ef")
                        nc.scalar.activation(r(eb[:]), gp[:], AF.Exp,
                                             bias=ynb[:, mt:mt + 1])
                        lz = r(ztf[:, ZP * mt:ZP * mt + ZP])
                        ebv = r(eb[:])
                    else:
                        eb = epool.tile([128, IC * ICW], BF, tag="eb")
                        nc.vector.tensor_scalar(eb[:].bitcast(I16), gp[:],
                                                float(EXP_S),
                                                ynb2[:, mt:mt + 1],
                                                op0=ALU.mult, op1=ALU.add)
                        lz = ztb[:, ZP * mt:ZP * mt + ZP]
                        ebv = eb[:]
                    for ic in range(IC):
                        nc.tensor.matmul(
                            accs[ic][0:ZP, :],
                            tile_position=(0, 0),
                            lhsT=lz,
                            rhs=ebv[:, ICW * ic:ICW * ic + ICW],
                            start=(k == 0), stop=(k == MT - 1),
                            skip_group_check=True)
                for ic in range(IC):
                    acc_s = finp.tile([ZP, ICW], FP, tag="accs")
                    nc.vector.tensor_copy(acc_s[:], accs[ic][0:ZP, :])
                    ot = apool.tile([128, 4 * ZP], FP, tag=f"ot{ic}")
                    for q in range(4):
                        nc.tensor.matmul(
                            ot[:, ZP * q:ZP * q + ZP],
                            tile_position=(0, 0),
                            lhsT=acc_s[0:ZP, 128 * q:128 * q + 128],
                            rhs=ident[0:ZP, 0:ZP],
                            is_transpose=True,
                            start=(q == 0), stop=(q == 3),
                            skip_group_check=True)
                    for q in range(4):
                        rec = finp.tile([128, 1], FP, tag="rec")
                        nc.vector.reciprocal(rec[:],
                                             ot[:, ZP * q + T:ZP * q + T + 1])
                        res = finp.tile([128, T], FP, tag="res")
                        nc.vector.tensor_scalar_mul(res[:],
                                                    ot[:, ZP * q:ZP * q + T],
                                                    rec[:])
                        row = 256 * q + 128 * ic
                        nc.sync.dma_start(OUTd[row:row + 128, :], res[:])
    nc.compile()
    return nc


def make_in_maps(X, Y, Y_target, W1, b1, W2, b2, W3, b3, n_cores=N_CORES):
    import ml_dtypes

    f = lambda a: np.ascontiguousarray(np.asarray(a, dtype=np.float32))
    X, Y, Y_target = f(X), f(Y), f(Y_target)
    W1, W2, W3 = f(W1), f(W2), f(W3)
    b1, b2, b3 = f(b1), f(b2), f(b3)
    m_total = Y.shape[0]
    n_sh = X.shape[0] // n_cores
    Zf = np.zeros((m_total, ZP), np.float32)
    Zf[:, :T] = Y_target
    Zf[:, T] = 1.0
    Bs = np.stack([np.tile(b1, 4), np.tile(b2, 4), np.tile(b3, 4)], axis=1)
    common = dict(
        Y=Y, Zf=Zf, Zb=Zf.astype(ml_dtypes.bfloat16),
        W1=W1, W2=W2, W3=W3,
        Bs=np.ascontiguousarray(Bs),
        ident=np.eye(128, dtype=np.float32),
    )
    return [dict(common, X=X[c * n_sh:(c + 1) * n_sh]) for c in range(n_cores)]


_NC_CACHE = {}


def _get_nc(n_sh, m_total):
    key = (n_sh, m_total)
    if key not in _NC_CACHE:
        use_f32r = os.environ.get("DKR_F32R", "1") == "1"
        _NC_CACHE[key] = build_nc(n_sh, m_total, use_f32r=use_f32r)
    return _NC_CACHE[key]


def kernel(X, Y, Y_target, W1, b1, W2, b2, W3, b3):
    from concourse.bass_utils import run_bass_kernel_spmd

    in_maps = make_in_maps(X, Y, Y_target, W1, b1, W2, b2, W3, b3)
    n_sh = in_maps[0]["X"].shape[0]
    nc = _get_nc(n_sh, np.asarray(Y).shape[0])
    res = run_bass_kernel_spmd(nc, in_maps, core_ids=list(range(N_CORES)))
    return np.concatenate([res.results[c]["out"] for c in range(N_CORES)], axis=0)


# revision 23
# speedup vs baseline: 3865.0204x; 1.1959x over previous
"""Trainium2 Bass kernel for DeepKernelRegressionModel (v2).

Math (per core, X sharded by rows across 8 cores):
  Xf = MLP(X), Yf = MLP(Y)                 (3-layer relu MLP, H=32)
  K[i,m] = exp(Xf_i . Yf_m - |Yf_m|^2/2)   (x-norm term cancels in the
                                            normalized ratio, so skip it)
  out = (K @ Y_target) / (K @ 1)

Design:
  - Y loaded in 8 batched DMAs as [128, 512] tiles, PE-transposed two
    m-tiles per transpose into yT [128, 4096] (even tiles on partitions
    0-63, odd on 64-127).
  - Y-MLP runs 4-way stacked; its relu output yfs [128, 2048] is read
    DIRECTLY as mm1's stationary operand (no assembled yft, no SP DMA
    assembly).
  - y-norms are computed as per-m-tile [128,1] bias COLUMNS via tiny
    matmuls (sqy-slice^T @ -0.5) and applied as the exp activation bias.
  - exp engine per m-tile is configurable: 'A' = exact exp on ACT (f32
    output), 'P'/'D' = Schraudolph bf16 bit-trick on Pool/DVE (one
    tensor_scalar op writing int16 bits of a bf16 exp approximation).
  - mm2 contracts exp tiles with Z = [Y_target, 1, pad] (ZP=16 cols) in
    matching dtype (f32r or bf16); m-tiles are visited in an order that
    rotates mm1 across all 4 PE row-groups for tile concurrency.
"""

import os
import numpy as np
from contextlib import ExitStack

import concourse.bass as bass
import concourse.tile as tile
from concourse import bacc, mybir

FP = mybir.dt.float32
FPR = mybir.dt.float32r
BF = mybir.dt.bfloat16
I16 = mybir.dt.int16
AF = mybir.ActivationFunctionType
ALU = mybir.AluOpType

D, H, T = 64, 32, 8
ZP = 16          # Z columns: Y_target(8) + ones + pad
N_CORES = 8

LN2 = 0.6931471805599453
EXP_S = 128.0 / LN2          # bf16 schraudolph scale
SIGMA = 0.058
EXP_B = (127.0 - SIGMA) * 128.0

# exp-engine pattern over reordered m-tile position (period 8):
# 'A' exact ACT, 'P' Pool bit-trick, 'D' DVE bit-trick
PATTERN = os.environ.get("DKR_PATTERN", "AAAAAAAA")


def mt_order(MT):
    """Visit order rotating mm1 row-groups 0,1,2,3. Octet pair (16 tiles):
    [16a+2j, 16a+2j+1, 16a+8+2j, 16a+8+2j+1] has cg 0,1,2,3."""
    order = []
    a = 0
    while 16 * a < MT:
        hi = 16 * a + 8 < MT
        for j in range(4):
            order.append(16 * a + 2 * j)
            order.append(16 * a + 2 * j + 1)
            if hi:
                order.append(16 * a + 8 + 2 * j)
                order.append(16 * a + 8 + 2 * j + 1)
        a += 1
    assert sorted(order) == list(range(MT))
    return order


def build_nc(n_sh, m_total, use_f32r=True, pattern=None, iters=1,
             split_waits=True):
    assert n_sh % 1024 == 0 and m_total % 2048 == 0
    MT = m_total // 128     # m-tiles
    NCH = m_total // 512    # MLP chunks
    CCY = NCH // 4
    NYD = m_total // 1024   # batched Y DMAs
    IC = n_sh // 512
    ICW = 512
    NXT = n_sh // 128       # x tiles
    pattern = pattern or PATTERN

    def r(ap):
        return ap.bitcast(FPR) if use_f32r else ap

    nc = bacc.Bacc("TRN2", target_bir_lowering=False, debug=False,
                   num_devices=N_CORES)

    Xd = nc.dram_tensor("X", [n_sh, D], FP, kind="ExternalInput").ap()
    Yd = nc.dram_tensor("Y", [m_total, D], FP, kind="ExternalInput").ap()
    Zfd = nc.dram_tensor("Zf", [m_total, ZP], FP, kind="ExternalInput").ap()
    Zbd = nc.dram_tensor("Zb", [m_total, ZP], BF, kind="ExternalInput").ap()
    WBd = nc.dram_tensor("WB", [128, 227], FP, kind="ExternalInput").ap()
    OUTd = nc.dram_tensor("out", [n_sh, T], FP, kind="ExternalOutput").ap()

    with tile.TileContext(nc) as tc, ExitStack() as octx:
        loop_cm = tc.For_i(0, iters, name="bench") if iters > 1 else None
        if loop_cm is not None:
            octx.enter_context(loop_cm)
        with ExitStack() as ctx:
            const = ctx.enter_context(tc.tile_pool(name="const", bufs=1))
            big = ctx.enter_context(tc.tile_pool(name="big", bufs=1))

            wb = const.tile([128, 227], FP)
            nc.sync.dma_start(r(wb[:]), r(WBd[:]))
            w1s = wb[:, 0:32]
            w2s = wb[:, 32:64]
            w3s = wb[:, 64:96]
            bs = wb[:, 96:99]
            ident = wb[:, 99:227]
            nh = const.tile([128, 1], FP)
            nc.gpsimd.memset(nh[:], -0.5)

            ztf = const.tile([128, MT * ZP], FP)
            nc.scalar.dma_start(
                r(ztf.rearrange("p (t c) -> p t c", c=ZP)),
                r(Zfd.rearrange("(t p) c -> p t c", p=128)),
            )
            use_bf = any(c != "A" for c in pattern)
            ztb = None
            if use_bf:
                ztb = const.tile([128, MT * ZP], BF)
                nc.scalar.dma_start(
                    ztb.rearrange("p (t c) -> p t c", c=ZP),
                    Zbd.rearrange("(t p) c -> p t c", p=128),
                )

            yT = big.tile([128, m_total // 2], FP)   # packed transposed Y
            xT = big.tile([128, n_sh // 2], FP)
            yfs = big.tile([128, m_total // 4], FP)  # MLP(Y)^T, 4-way stacked
            xft = big.tile([128, n_sh], FP)          # MLP(X)^T, replicated x4
            ynb = big.tile([128, MT], FP)            # -|Yf|^2/2 bias columns
            ynb2 = big.tile([128, MT], FP)           # scaled for bit-trick

            # ---------- phase A: load + transpose (+ X MLP early) ----------
            with (
                tc.tile_pool(name="tp_psum", bufs=2, space="PSUM") as tpp,
                tc.tile_pool(name="raw", bufs=2) as rawp,
                tc.tile_pool(name="xp", bufs=2, space="PSUM") as xpp,
                tc.tile_pool(name="xp3", bufs=1, space="PSUM") as xpp3,
                tc.tile_pool(name="xacts", bufs=2) as xactp,
            ):
                xraw = rawp.tile([128, 512], FP, tag="xraw")
                nc.sync.dma_start(
                    r(xraw.rearrange("p (t c) -> p t c", c=D)),
                    r(Xd.rearrange("(t p) c -> p t c", p=128)),
                )
                tp = tpp.tile([128, 512], FP, tag="tp")
                for j in range(4):
                    nc.tensor.transpose(r(tp[:, 128 * j:128 * j + 128]),
                                        r(xraw[:, 128 * j:128 * j + 128]),
                                        r(ident))
                nc.vector.tensor_copy(r(xT[:]), tp[:])

                for g in range(NYD // 2):
                    yraw = rawp.tile([128, 1024], FP, tag="raw")
                    nc.sync.dma_start(
                        r(yraw.rearrange("p (t c) -> p t c", c=D)),
                        r(Yd[2048 * g:2048 * g + 2048, :].rearrange(
                            "(t p) c -> p t c", p=128)),
                    )
                    for h in range(2):
                        tp = tpp.tile([128, 512], FP, tag="tp")
                        for j in range(4):
                            nc.tensor.transpose(
                                r(tp[:, 128 * j:128 * j + 128]),
                                r(yraw[:, 512 * h + 128 * j:
                                        512 * h + 128 * j + 128]),
                                r(ident))
                        nc.vector.tensor_copy(
                            r(yT[:, 1024 * g + 512 * h:1024 * g + 512 * h + 512]),
                            tp[:])

                # X MLP (f32r, flat rows 0-31), interleaved with Y loads
                hx1 = xpp.tile([H, n_sh], FP, tag="hx")
                for half in range(2):
                    nc.tensor.matmul(
                        hx1[0:32, 512 * half:512 * half + 512],
                        tile_position=(64 * half, 0),
                        lhsT=r(w1s[64 * half:64 * half + 64, :]),
                        rhs=r(xT[64 * half:64 * half + 64, :]),
                        start=True, stop=True, skip_group_check=True)
                hx1s = xactp.tile([H, n_sh], FP, tag="hxs")
                nc.scalar.activation(r(hx1s[:]), hx1[:], AF.Relu,
                                      bias=bs[0:H, 0:1])
                hx2 = xpp.tile([H, n_sh], FP, tag="hx")
                for half in range(2):
                    nc.tensor.matmul(
                        hx2[0:32, 512 * half:512 * half + 512],
                        tile_position=(0, 0),
                        lhsT=r(w2s[0:32, :]),
                        rhs=r(hx1s[0:32, 512 * half:512 * half + 512]),
                        start=True, stop=True, skip_group_check=True)
                hx2s = xactp.tile([H, n_sh], FP, tag="hxs")
                nc.vector.tensor_scalar(r(hx2s[:]), hx2[:], bs[0:H, 1:2], 0.0,
                                        op0=ALU.add, op1=ALU.max)
                hx3 = xpp3.tile([H, n_sh], FP, tag="hx3")
                for half in range(2):
                    nc.tensor.matmul(
                        hx3[0:32, 512 * half:512 * half + 512],
                        tile_position=(0, 0),
                        lhsT=r(w3s[0:32, :]),
                        rhs=r(hx2s[0:32, 512 * half:512 * half + 512]),
                        start=True, stop=True, skip_group_check=True)
                nc.vector.tensor_scalar(r(xft[0:32, :]), hx3[0:32, :],
                                        bs[0:H, 2:3], 0.0,
                                        op0=ALU.add, op1=ALU.max)
                for gg in range(1, 4):
                    nc.gpsimd.dma_start(r(xft[32 * gg:32 * gg + 32, :]),
                                        r(xft[0:32, :]))

            def yfs_slice(mt):
                ch = 2 * (mt // 8) + (mt % 8) % 2
                j = (mt % 8) // 2
                cg, cc = ch % 4, ch // 4
                col = 512 * cc + 128 * j
                return cg, yfs[32 * cg:32 * cg + 32, col:col + 128]

            def sqy_slice(mt):
                ch = 2 * (mt // 8) + (mt % 8) % 2
                j = (mt % 8) // 2
                cg, cc = ch % 4, ch // 4
                col = 512 * cc + 128 * j
                return cg, sqy[32 * cg:32 * cg + 32, col:col + 128]

            # ---------- phase B: Y MLP ----------
            # L1/L2 are f32r, which the ISA only allows at column-group 0,
            # so they emit flat [32, m] rows 0-31. L3 is plain fp32 (legal
            # with column groups) and emits the 4-way partition-stacked yfs
            # that mm1's rotating row-groups read directly.
            sqyp = ctx.enter_context(tc.tile_pool(name="sqy", bufs=1))
            with (
                tc.tile_pool(name="mlp_psum", bufs=2, space="PSUM") as mpp,
                tc.tile_pool(name="l3_psum", bufs=2, space="PSUM") as mpp3,
                tc.tile_pool(name="ynp", bufs=2, space="PSUM") as ynpp,
                tc.tile_pool(name="acts", bufs=1) as actp,
            ):
                h1s = actp.tile([H, m_total], FP, tag="h1s")
                h2s = actp.tile([H, m_total], FP, tag="h2s")
                npass = (NCH + 1) // 2
                for p in range(npass):
                    chs = range(2 * p, min(2 * p + 2, NCH))
                    h1p = mpp.tile([H, 1024], FP, tag="hp")
                    for i, ch in enumerate(chs):
                        q, half = ch // 2, ch % 2
                        nc.tensor.matmul(
                            h1p[:, 512 * i:512 * i + 512],
                            lhsT=r(w1s[64 * half:64 * half + 64, :]),
                            rhs=r(yT[64 * half:64 * half + 64,
                                     512 * q:512 * q + 512]),
                            tile_position=(64 * half, 0),
                            start=True, stop=True, skip_group_check=True)
                    nc.scalar.activation(
                        r(h1s[:, 1024 * p:1024 * p + 512 * len(chs)]),
                        h1p[:, 0:512 * len(chs)], AF.Relu, bias=bs[0:H, 0:1])
                for p in range(npass):
                    chs = range(2 * p, min(2 * p + 2, NCH))
                    h2p = mpp.tile([H, 1024], FP, tag="hp")
                    for i, ch in enumerate(chs):
                        nc.tensor.matmul(
                            h2p[:, 512 * i:512 * i + 512],
                            lhsT=r(w2s[0:32, :]),
                            rhs=r(h1s[0:32, 512 * ch:512 * ch + 512]),
                            tile_position=(0, 0),
                            start=True, stop=True, skip_group_check=True)
                    nc.vector.tensor_scalar(
                        r(h2s[:, 1024 * p:1024 * p + 512 * len(chs)]),
                        h2p[:, 0:512 * len(chs)], bs[0:H, 1:2], 0.0,
                        op0=ALU.add, op1=ALU.max)
                # L3: fp32, col-grouped into the stacked layout, per-cc
                sqy = sqyp.tile([128, 512 * CCY], FP, tag="sqy")
                for cc in range(CCY):
                    h3p = mpp3.tile([128, 512], FP, tag="h3p")
                    for cg in range(4):
                        ch = 4 * cc + cg
                        nc.tensor.matmul(
                            h3p[32 * cg:32 * cg + 32, :],
                            tile_position=(0, 32 * cg),
                            lhsT=w3s[0:32, :],
                            rhs=h2s[0:32, 512 * ch:512 * ch + 512],
                            start=True, stop=True, skip_group_check=True)
                    nc.vector.tensor_scalar(
                        r(yfs[:, 512 * cc:512 * cc + 512]),
                        h3p[:], bs[:, 2:3], 0.0, op0=ALU.add, op1=ALU.max)
                    nc.vector.tensor_mul(sqy[:, 512 * cc:512 * cc + 512],
                                         yfs[:, 512 * cc:512 * cc + 512],
                                         yfs[:, 512 * cc:512 * cc + 512])
                    # norm-bias columns for this cc's 16 m-tiles
                    ynp = ynpp.tile([128, 16], FP, tag="ynp")
                    mts = [mt for mt in range(16 * cc, min(16 * cc + 16, MT))]
                    for kk, mt in enumerate(mts):
                        scg, sl = sqy_slice(mt)
                        nc.tensor.matmul(
                            ynp[:, kk:kk + 1],
                            tile_position=(32 * scg, 0),
                            lhsT=sl, rhs=nh[32 * scg:32 * scg + 32, :],
                            start=True, stop=True, skip_group_check=True)
                    nc.vector.tensor_copy(ynb[:, 16 * cc:16 * cc + len(mts)],
                                          ynp[:, 0:len(mts)])

            if use_bf:
                nc.vector.tensor_scalar(ynb2[:], ynb[:], float(EXP_S),
                                        float(EXP_B), op0=ALU.mult,
                                        op1=ALU.add)

            # ---------- main loop ----------
            # Both i-chunks processed per m-tile: one [128, 1024] exp per
            # tile (single bias), one weight load per mm1 pair, and two
            # long-lived accumulators.
            order = mt_order(MT)
            with (
                tc.tile_pool(name="gbuf", bufs=2, space="PSUM") as gpool,
                tc.tile_pool(name="accp", bufs=1, space="PSUM") as apool,
                tc.tile_pool(name="ebuf", bufs=3) as epool,
                tc.tile_pool(name="fin", bufs=2) as finp,
            ):
                accs = []
                for ic in range(IC):
                    acc = apool.tile([128, ICW], FP, tag=f"acc{ic}")
                    accs.append(acc)
                for k, mt in enumerate(order):
                    eng = pattern[k % len(pattern)]
                    cg, ysl = yfs_slice(mt)
                    gp = gpool.tile([128, IC * ICW], FP, tag="g")
                    for ic in range(IC):
                        nc.tensor.matmul(
                            gp[:, ICW * ic:ICW * ic + ICW],
                            tile_position=(32 * cg, 0),
                            lhsT=r(ysl),
                            rhs=r(xft[32 * cg:32 * cg + 32,
                                      ICW * ic:ICW * ic + ICW]),
                            start=True, stop=True, skip_group_check=True)
                    if eng == "A":
                        eb = epool.tile([128, IC * ICW], FP, tag="ef")
                        nc.scalar.activation(r(eb[:]), gp[:], AF.Exp,
                                             bias=ynb[:, mt:mt + 1])
                        lz = r(ztf[:, ZP * mt:ZP * mt + ZP])
                        ebv = r(eb[:])
                    else:
                        eb = epool.tile([128, IC * ICW], BF, tag="eb")
                        nc.vector.tensor_scalar(eb[:].bitcast(I16), gp[:],
                                                float(EXP_S),
                                                ynb2[:, mt:mt + 1],
                                                op0=ALU.mult, op1=ALU.add)
                        lz = ztb[:, ZP * mt:ZP * mt + ZP]
                        ebv = eb[:]
                    for ic in range(IC):
                        nc.tensor.matmul(
                            accs[ic][0:ZP, :],
                            tile_position=(0, 0),
                            lhsT=lz,
                            rhs=ebv[:, ICW * ic:ICW * ic + ICW],
                            start=(k == 0), stop=(k == MT - 1),
                            skip_group_check=True)
                for ic in range(IC):
                    acc_s = finp.tile([ZP, ICW], FP, tag="accs")
                    nc.vector.tensor_copy(acc_s[:], accs[ic][0:ZP, :])
                    ot = apool.tile([128, 4 * ZP], FP, tag=f"ot{ic}")
                    for q in range(4):
                        nc.tensor.matmul(
                            ot[:, ZP * q:ZP * q + ZP],
                            tile_position=(0, 0),
                            lhsT=acc_s[0:ZP, 128 * q:128 * q + 128],
                            rhs=ident[0:ZP, 0:ZP],
                            is_transpose=True,
                            start=(q == 0), stop=(q == 3),
                            skip_group_check=True)
                    for q in range(4):
                        rec = finp.tile([128, 1], FP, tag="rec")
                        nc.vector.reciprocal(rec[:],
                                             ot[:, ZP * q + T:ZP * q + T + 1])
                        res = finp.tile([128, T], FP, tag="res")
                        nc.vector.tensor_scalar_mul(res[:],
                                                    ot[:, ZP * q:ZP * q + T],
                                                    rec[:])
                        row = 256 * q + 128 * ic
                        nc.sync.dma_start(OUTd[row:row + 128, :], res[:])
    nc.compile()
    return nc


def make_in_maps(X, Y, Y_target, W1, b1, W2, b2, W3, b3, n_cores=N_CORES):
    import ml_dtypes

    f = lambda a: np.ascontiguousarray(np.asarray(a, dtype=np.float32))
    X, Y, Y_target = f(X), f(Y), f(Y_target)
    W1, W2, W3 = f(W1), f(W2), f(W3)
    b1, b2, b3 = f(b1), f(b2), f(b3)
    m_total = Y.shape[0]
    n_sh = X.shape[0] // n_cores
    Zf = np.zeros((m_total, ZP), np.float32)
    Zf[:, :T] = Y_target
    Zf[:, T] = 1.0
    WB = np.zeros((128, 227), np.float32)
    WB[0:64, 0:32] = W1
    WB[64:128, 0:32] = W1
    WB[:, 32:64] = np.tile(W2, (4, 1))
    WB[:, 64:96] = np.tile(W3, (4, 1))
    WB[:, 96] = np.tile(b1, 4)
    WB[:, 97] = np.tile(b2, 4)
    WB[:, 98] = np.tile(b3, 4)
    WB[:, 99:227] = np.eye(128, dtype=np.float32)
    common = dict(
        Y=Y, Zf=Zf, Zb=Zf.astype(ml_dtypes.bfloat16),
        WB=np.ascontiguousarray(WB),
    )
    return [dict(common, X=X[c * n_sh:(c + 1) * n_sh]) for c in range(n_cores)]


_NC_CACHE = {}


def _get_nc(n_sh, m_total):
    key = (n_sh, m_total)
    if key not in _NC_CACHE:
        use_f32r = os.environ.get("DKR_F32R", "1") == "1"
        _NC_CACHE[key] = build_nc(n_sh, m_total, use_f32r=use_f32r)
    return _NC_CACHE[key]


def kernel(X, Y, Y_target, W1, b1, W2, b2, W3, b3):
    from concourse.bass_utils import run_bass_kernel_spmd

    in_maps = make_in_maps(X, Y, Y_target, W1, b1, W2, b2, W3, b3)
    n_sh = in_maps[0]["X"].shape[0]
    nc = _get_nc(n_sh, np.asarray(Y).shape[0])
    res = run_bass_kernel_spmd(nc, in_maps, core_ids=list(range(N_CORES)))
    return np.concatenate([res.results[c]["out"] for c in range(N_CORES)], axis=0)


# revision 29
# speedup vs baseline: 4051.5897x; 1.0483x over previous
"""Trainium2 Bass kernel for DeepKernelRegressionModel (v2).

Math (per core, X sharded by rows across 8 cores):
  Xf = MLP(X), Yf = MLP(Y)                 (3-layer relu MLP, H=32)
  K[i,m] = exp(Xf_i . Yf_m - |Yf_m|^2/2)   (x-norm term cancels in the
                                            normalized ratio, so skip it)
  out = (K @ Y_target) / (K @ 1)

Design:
  - Y loaded in 8 batched DMAs as [128, 512] tiles, PE-transposed two
    m-tiles per transpose into yT [128, 4096] (even tiles on partitions
    0-63, odd on 64-127).
  - Y-MLP runs 4-way stacked; its relu output yfs [128, 2048] is read
    DIRECTLY as mm1's stationary operand (no assembled yft, no SP DMA
    assembly).
  - y-norms are computed as per-m-tile [128,1] bias COLUMNS via tiny
    matmuls (sqy-slice^T @ -0.5) and applied as the exp activation bias.
  - exp engine per m-tile is configurable: 'A' = exact exp on ACT (f32
    output), 'P'/'D' = Schraudolph bf16 bit-trick on Pool/DVE (one
    tensor_scalar op writing int16 bits of a bf16 exp approximation).
  - mm2 contracts exp tiles with Z = [Y_target, 1, pad] (ZP=16 cols) in
    matching dtype (f32r or bf16); m-tiles are visited in an order that
    rotates mm1 across all 4 PE row-groups for tile concurrency.
"""

import os
import numpy as np
from contextlib import ExitStack

import concourse.bass as bass
import concourse.tile as tile
from concourse import bacc, mybir

FP = mybir.dt.float32
FPR = mybir.dt.float32r
BF = mybir.dt.bfloat16
I16 = mybir.dt.int16
AF = mybir.ActivationFunctionType
ALU = mybir.AluOpType

D, H, T = 64, 32, 8
ZP = 16          # Z columns: Y_target(8) + ones + pad
N_CORES = 8

LN2 = 0.6931471805599453
EXP_S = 128.0 / LN2          # bf16 schraudolph scale
SIGMA = 0.058
EXP_B = (127.0 - SIGMA) * 128.0

# exp-engine pattern over reordered m-tile position (period 8):
# 'A' exact ACT, 'P' Pool bit-trick, 'D' DVE bit-trick
PATTERN = os.environ.get("DKR_PATTERN", "AAAAAAAA")


def mt_order(MT):
    """Visit order rotating mm1 row-groups 0,1,2,3. Octet pair (16 tiles):
    [16a+2j, 16a+2j+1, 16a+8+2j, 16a+8+2j+1] has cg 0,1,2,3."""
    order = []
    a = 0
    while 16 * a < MT:
        hi = 16 * a + 8 < MT
        for j in range(4):
            order.append(16 * a + 2 * j)
            order.append(16 * a + 2 * j + 1)
            if hi:
                order.append(16 * a + 8 + 2 * j)
                order.append(16 * a + 8 + 2 * j + 1)
        a += 1
    assert sorted(order) == list(range(MT))
    return order


def build_nc(n_sh, m_total, use_f32r=True, pattern=None, iters=1,
             split_waits=True):
    assert n_sh % 1024 == 0 and m_total % 2048 == 0
    MT = m_total // 128     # m-tiles
    NCH = m_total // 512    # MLP chunks
    CCY = NCH // 4
    NYD = m_total // 1024   # batched Y DMAs
    IC = n_sh // 512
    ICW = 512
    NXT = n_sh // 128       # x tiles
    pattern = pattern or PATTERN

    def r(ap):
        return ap.bitcast(FPR) if use_f32r else ap

    nc = bacc.Bacc("TRN2", target_bir_lowering=False, debug=False,
                   num_devices=N_CORES)

    Xd = nc.dram_tensor("X", [n_sh, D], FP, kind="ExternalInput").ap()
    Yd = nc.dram_tensor("Y", [m_total, D], FP, kind="ExternalInput").ap()
    Zfd = nc.dram_tensor("Zf", [m_total, ZP], FP, kind="ExternalInput").ap()
    Zbd = nc.dram_tensor("Zb", [m_total, ZP], BF, kind="ExternalInput").ap()
    WBd = nc.dram_tensor("WB", [128, 227], FP, kind="ExternalInput").ap()
    OUTd = nc.dram_tensor("out", [n_sh, T], FP, kind="ExternalOutput").ap()

    with tile.TileContext(nc) as tc, ExitStack() as octx:
        loop_cm = tc.For_i(0, iters, name="bench") if iters > 1 else None
        if loop_cm is not None:
            octx.enter_context(loop_cm)
        with ExitStack() as ctx:
            const = ctx.enter_context(tc.tile_pool(name="const", bufs=1))
            big = ctx.enter_context(tc.tile_pool(name="big", bufs=1))

            wb = const.tile([128, 227], FP)
            nc.sync.dma_start(r(wb[:]), r(WBd[:]))
            w1s = wb[:, 0:32]
            w2s = wb[:, 32:64]
            w3s = wb[:, 64:96]
            bs = wb[:, 96:99]
            ident = wb[:, 99:227]
            nh = const.tile([128, 1], FP)
            nc.gpsimd.memset(nh[:], -0.5)

            ztf = const.tile([128, MT * ZP], FP)
            nc.gpsimd.dma_start(
                r(ztf.rearrange("p (t c) -> p t c", c=ZP)),
                r(Zfd.rearrange("(t p) c -> p t c", p=128)),
            )
            use_bf = any(c != "A" for c in pattern)
            ztb = None
            if use_bf:
                ztb = const.tile([128, MT * ZP], BF)
                nc.scalar.dma_start(
                    ztb.rearrange("p (t c) -> p t c", c=ZP),
                    Zbd.rearrange("(t p) c -> p t c", p=128),
                )

            yT = big.tile([128, m_total // 2], FP)   # packed transposed Y
            xT = big.tile([128, n_sh // 2], FP)
            yfs = big.tile([128, m_total // 4], FP)  # MLP(Y)^T, 4-way stacked
            xft = big.tile([128, n_sh], FP)          # MLP(X)^T, replicated x4
            ynb = big.tile([128, MT], FP)            # -|Yf|^2/2 bias columns
            ynb2 = big.tile([128, MT], FP)           # scaled for bit-trick

            # ---------- phase A: load + transpose (+ X MLP early) ----------
            with (
                tc.tile_pool(name="tp_psum", bufs=2, space="PSUM") as tpp,
                tc.tile_pool(name="raw", bufs=2) as rawp,
                tc.tile_pool(name="xp", bufs=2, space="PSUM") as xpp,
                tc.tile_pool(name="xp3", bufs=1, space="PSUM") as xpp3,
                tc.tile_pool(name="xacts", bufs=2) as xactp,
            ):
                xraw = rawp.tile([128, 512], FP, tag="xraw")
                nc.sync.dma_start(
                    r(xraw.rearrange("p (t c) -> p t c", c=D)),
                    r(Xd.rearrange("(t p) c -> p t c", p=128)),
                )
                tp = tpp.tile([128, 512], FP, tag="tp")
                for j in range(4):
                    nc.tensor.transpose(r(tp[:, 128 * j:128 * j + 128]),
                                        r(xraw[:, 128 * j:128 * j + 128]),
                                        r(ident))
                nc.vector.tensor_copy(r(xT[:]), tp[:])

                dma_engines = [nc.sync, nc.scalar, nc.sync, nc.scalar]
                for g in range(NYD // 2):
                    yraw = rawp.tile([128, 1024], FP, tag="raw")
                    dma_engines[g % 4].dma_start(
                        r(yraw.rearrange("p (t c) -> p t c", c=D)),
                        r(Yd[2048 * g:2048 * g + 2048, :].rearrange(
                            "(t p) c -> p t c", p=128)),
                    )
                    for h in range(2):
                        tp = tpp.tile([128, 512], FP, tag="tp")
                        for j in range(4):
                            nc.tensor.transpose(
                                r(tp[:, 128 * j:128 * j + 128]),
                                r(yraw[:, 512 * h + 128 * j:
                                        512 * h + 128 * j + 128]),
                                r(ident))
                        nc.vector.tensor_copy(
                            r(yT[:, 1024 * g + 512 * h:1024 * g + 512 * h + 512]),
                            tp[:])

                # X MLP (f32r, flat rows 0-31), interleaved with Y loads
                hx1 = xpp.tile([H, n_sh], FP, tag="hx")
                for half in range(2):
                    nc.tensor.matmul(
                        hx1[0:32, 512 * half:512 * half + 512],
                        tile_position=(64 * half, 0),
                        lhsT=r(w1s[64 * half:64 * half + 64, :]),
                        rhs=r(xT[64 * half:64 * half + 64, :]),
                        start=True, stop=True, skip_group_check=True)
                hx1s = xactp.tile([H, n_sh], FP, tag="hxs")
                nc.scalar.activation(r(hx1s[:]), hx1[:], AF.Relu,
                                      bias=bs[0:H, 0:1])
                hx2 = xpp.tile([H, n_sh], FP, tag="hx")
                for half in range(2):
                    nc.tensor.matmul(
                        hx2[0:32, 512 * half:512 * half + 512],
                        tile_position=(0, 0),
                        lhsT=r(w2s[0:32, :]),
                        rhs=r(hx1s[0:32, 512 * half:512 * half + 512]),
                        start=True, stop=True, skip_group_check=True)
                hx2s = xactp.tile([H, n_sh], FP, tag="hxs")
                nc.vector.tensor_scalar(r(hx2s[:]), hx2[:], bs[0:H, 1:2], 0.0,
                                        op0=ALU.add, op1=ALU.max)
                hx3 = xpp3.tile([H, n_sh], FP, tag="hx3")
                for half in range(2):
                    nc.tensor.matmul(
                        hx3[0:32, 512 * half:512 * half + 512],
                        tile_position=(0, 0),
                        lhsT=r(w3s[0:32, :]),
                        rhs=r(hx2s[0:32, 512 * half:512 * half + 512]),
                        start=True, stop=True, skip_group_check=True)
                nc.vector.tensor_scalar(r(xft[0:32, :]), hx3[0:32, :],
                                        bs[0:H, 2:3], 0.0,
                                        op0=ALU.add, op1=ALU.max)
                for gg in range(1, 4):
                    nc.gpsimd.dma_start(r(xft[32 * gg:32 * gg + 32, :]),
                                        r(xft[0:32, :]))

            def yfs_slice(mt):
                ch = 2 * (mt // 8) + (mt % 8) % 2
                j = (mt % 8) // 2
                cg, cc = ch % 4, ch // 4
                col = 512 * cc + 128 * j
                return cg, yfs[32 * cg:32 * cg + 32, col:col + 128]

            def sqy_slice(mt):
                ch = 2 * (mt // 8) + (mt % 8) % 2
                j = (mt % 8) // 2
                cg, cc = ch % 4, ch // 4
                col = 512 * cc + 128 * j
                return cg, sqy[32 * cg:32 * cg + 32, col:col + 128]

            # ---------- phase B: Y MLP ----------
            # L1/L2 are f32r, which the ISA only allows at column-group 0,
            # so they emit flat [32, m] rows 0-31. L3 is plain fp32 (legal
            # with column groups) and emits the 4-way partition-stacked yfs
            # that mm1's rotating row-groups read directly.
            sqyp = ctx.enter_context(tc.tile_pool(name="sqy", bufs=1))
            with (
                tc.tile_pool(name="mlp_psum", bufs=2, space="PSUM") as mpp,
                tc.tile_pool(name="l3_psum", bufs=2, space="PSUM") as mpp3,
                tc.tile_pool(name="ynp", bufs=2, space="PSUM") as ynpp,
                tc.tile_pool(name="acts", bufs=1) as actp,
            ):
                h1s = actp.tile([H, m_total], FP, tag="h1s")
                h2s = actp.tile([H, m_total], FP, tag="h2s")
                npass = (NCH + 1) // 2
                for p in range(npass):
                    chs = range(2 * p, min(2 * p + 2, NCH))
                    h1p = mpp.tile([H, 1024], FP, tag="hp")
                    for i, ch in enumerate(chs):
                        q, half = ch // 2, ch % 2
                        nc.tensor.matmul(
                            h1p[:, 512 * i:512 * i + 512],
                            lhsT=r(w1s[64 * half:64 * half + 64, :]),
                            rhs=r(yT[64 * half:64 * half + 64,
                                     512 * q:512 * q + 512]),
                            tile_position=(64 * half, 0),
                            start=True, stop=True, skip_group_check=True)
                    nc.scalar.activation(
                        r(h1s[:, 1024 * p:1024 * p + 512 * len(chs)]),
                        h1p[:, 0:512 * len(chs)], AF.Relu, bias=bs[0:H, 0:1])
                for p in range(npass):
                    chs = range(2 * p, min(2 * p + 2, NCH))
                    h2p = mpp.tile([H, 1024], FP, tag="hp")
                    for i, ch in enumerate(chs):
                        nc.tensor.matmul(
                            h2p[:, 512 * i:512 * i + 512],
                            lhsT=r(w2s[0:32, :]),
                            rhs=r(h1s[0:32, 512 * ch:512 * ch + 512]),
                            tile_position=(0, 0),
                            start=True, stop=True, skip_group_check=True)
                    nc.vector.tensor_scalar(
                        r(h2s[:, 1024 * p:1024 * p + 512 * len(chs)]),
                        h2p[:, 0:512 * len(chs)], bs[0:H, 1:2], 0.0,
                        op0=ALU.add, op1=ALU.max)
                # L3: fp32, col-grouped into the stacked layout, per-cc
                sqy = sqyp.tile([128, 512 * CCY], FP, tag="sqy")
                for cc in range(CCY):
                    h3p = mpp3.tile([128, 512], FP, tag="h3p")
                    for cg in range(4):
                        ch = 4 * cc + cg
                        nc.tensor.matmul(
                            h3p[32 * cg:32 * cg + 32, :],
                            tile_position=(0, 32 * cg),
                            lhsT=w3s[0:32, :],
                            rhs=h2s[0:32, 512 * ch:512 * ch + 512],
                            start=True, stop=True, skip_group_check=True)
                    nc.vector.tensor_scalar(
                        r(yfs[:, 512 * cc:512 * cc + 512]),
                        h3p[:], bs[:, 2:3], 0.0, op0=ALU.add, op1=ALU.max)
                    nc.vector.tensor_mul(sqy[:, 512 * cc:512 * cc + 512],
                                         yfs[:, 512 * cc:512 * cc + 512],
                                         yfs[:, 512 * cc:512 * cc + 512])
                    ynp = ynpp.tile([128, 16], FP, tag="ynp")
                    mts = [mt for mt in range(16 * cc, min(16 * cc + 16, MT))]
                    for kk, mt in enumerate(mts):
                        scg, sl = sqy_slice(mt)
                        nc.tensor.matmul(
                            ynp[:, kk:kk + 1],
                            tile_position=(32 * scg, 0),
                            lhsT=sl, rhs=nh[32 * scg:32 * scg + 32, :],
                            start=True, stop=True, skip_group_check=True)
                    nc.vector.tensor_copy(ynb[:, 16 * cc:16 * cc + len(mts)],
                                          ynp[:, 0:len(mts)])

            if use_bf:
                nc.vector.tensor_scalar(ynb2[:], ynb[:], float(EXP_S),
                                        float(EXP_B), op0=ALU.mult,
                                        op1=ALU.add)

            # ---------- main loop ----------
            # Both i-chunks processed per m-tile: one [128, 1024] exp per
            # tile (single bias), one weight load per mm1 pair, and two
            # long-lived accumulators.
            order = mt_order(MT)
            with (
                tc.tile_pool(name="gbuf", bufs=2, space="PSUM") as gpool,
                tc.tile_pool(name="accp", bufs=1, space="PSUM") as apool,
                tc.tile_pool(name="ebuf", bufs=3) as epool,
                tc.tile_pool(name="fin", bufs=2) as finp,
            ):
                accs = []
                for ic in range(IC):
                    acc = apool.tile([128, ICW], FP, tag=f"acc{ic}")
                    accs.append(acc)
                for k, mt in enumerate(order):
                    eng = pattern[k % len(pattern)]
                    cg, ysl = yfs_slice(mt)
                    gp = gpool.tile([128, IC * ICW], FP, tag="g")
                    for ic in range(IC):
                        nc.tensor.matmul(
                            gp[:, ICW * ic:ICW * ic + ICW],
                            tile_position=(32 * cg, 0),
                            lhsT=r(ysl),
                            rhs=r(xft[32 * cg:32 * cg + 32,
                                      ICW * ic:ICW * ic + ICW]),
                            start=True, stop=True, skip_group_check=True)
                    if eng == "A":
                        eb = epool.tile([128, IC * ICW], FP, tag="ef")
                        nc.scalar.activation(r(eb[:]), gp[:], AF.Exp,
                                             bias=ynb[:, mt:mt + 1])
                        lz = r(ztf[:, ZP * mt:ZP * mt + ZP])
                        ebv = r(eb[:])
                    else:
                        eb = epool.tile([128, IC * ICW], BF, tag="eb")
                        nc.vector.tensor_scalar(eb[:].bitcast(I16), gp[:],
                                                float(EXP_S),
                                                ynb2[:, mt:mt + 1],
                                                op0=ALU.mult, op1=ALU.add)
                        lz = ztb[:, ZP * mt:ZP * mt + ZP]
                        ebv = eb[:]
                    for ic in range(IC):
                        nc.tensor.matmul(
                            accs[ic][0:ZP, :],
                            tile_position=(0, 0),
                            lhsT=lz,
                            rhs=ebv[:, ICW * ic:ICW * ic + ICW],
                            start=(k == 0), stop=(k == MT - 1),
                            skip_group_check=True)
                otm = apool.tile([128, 8 * ZP], FP, tag="ot")
                for ic in range(IC):
                    acc_s = finp.tile([ZP, ICW], FP, tag="accs")
                    nc.vector.tensor_copy(acc_s[:], accs[ic][0:ZP, :])
                    ot = otm[:, 4 * ZP * ic:4 * ZP * ic + 4 * ZP]
                    for q in range(4):
                        nc.tensor.matmul(
                            ot[:, ZP * q:ZP * q + ZP],
                            tile_position=(0, 0),
                            lhsT=acc_s[0:ZP, 128 * q:128 * q + 128],
                            rhs=ident[0:ZP, 0:ZP],
                            is_transpose=True,
                            start=(q == 0), stop=(q == 3),
                            skip_group_check=True)
                    for q in range(4):
                        rec = finp.tile([128, 1], FP, tag="rec")
                        nc.vector.reciprocal(rec[:],
                                             ot[:, ZP * q + T:ZP * q + T + 1])
                        res = finp.tile([128, T], FP, tag="res")
                        nc.vector.tensor_scalar_mul(res[:],
                                                    ot[:, ZP * q:ZP * q + T],
                                                    rec[:])
                        row = 256 * q + 128 * ic
                        (nc.sync if ic == 0 else nc.gpsimd).dma_start(
                            OUTd[row:row + 128, :], res[:])
    nc.compile()
    return nc


def make_in_maps(X, Y, Y_target, W1, b1, W2, b2, W3, b3, n_cores=N_CORES):
    import ml_dtypes

    f = lambda a: np.ascontiguousarray(np.asarray(a, dtype=np.float32))
    X, Y, Y_target = f(X), f(Y), f(Y_target)
    W1, W2, W3 = f(W1), f(W2), f(W3)
    b1, b2, b3 = f(b1), f(b2), f(b3)
    m_total = Y.shape[0]
    n_sh = X.shape[0] // n_cores
    Zf = np.zeros((m_total, ZP), np.float32)
    Zf[:, :T] = Y_target
    Zf[:, T] = 1.0
    WB = np.zeros((128, 227), np.float32)
    WB[0:64, 0:32] = W1
    WB[64:128, 0:32] = W1
    WB[:, 32:64] = np.tile(W2, (4, 1))
    WB[:, 64:96] = np.tile(W3, (4, 1))
    WB[:, 96] = np.tile(b1, 4)
    WB[:, 97] = np.tile(b2, 4)
    WB[:, 98] = np.tile(b3, 4)
    WB[:, 99:227] = np.eye(128, dtype=np.float32)
    common = dict(
        Y=Y, Zf=Zf, Zb=Zf.astype(ml_dtypes.bfloat16),
        WB=np.ascontiguousarray(WB),
    )
    return [dict(common, X=X[c * n_sh:(c + 1) * n_sh]) for c in range(n_cores)]


_NC_CACHE = {}


def _get_nc(n_sh, m_total):
    key = (n_sh, m_total)
    if key not in _NC_CACHE:
        use_f32r = os.environ.get("DKR_F32R", "1") == "1"
        _NC_CACHE[key] = build_nc(n_sh, m_total, use_f32r=use_f32r)
    return _NC_CACHE[key]


def kernel(X, Y, Y_target, W1, b1, W2, b2, W3, b3):
    from concourse.bass_utils import run_bass_kernel_spmd

    in_maps = make_in_maps(X, Y, Y_target, W1, b1, W2, b2, W3, b3)
    n_sh = in_maps[0]["X"].shape[0]
    nc = _get_nc(n_sh, np.asarray(Y).shape[0])
    res = run_bass_kernel_spmd(nc, in_maps, core_ids=list(range(N_CORES)))
    return np.concatenate([res.results[c]["out"] for c in range(N_CORES)], axis=0)
